# revision 3
# baseline (speedup 1.0000x reference)
"""Trainium2 Bass kernel for nn_FCGF_point_att3 (segment_reduce).

Pipeline (per reference.py):
  h = x@W1.T + b1 ; h = relu(BN(h)) ; a = BN(h@W2.T + b2)
  out = l2norm(segment_mean(x * a))   with global (all-N) BN stats.

Strategy: 8-way data parallel over segments (2 segments of 50k points per
core).  Two SPMD launches:
  L1: per-core Gram matrix G = [X|1]^T[X|1] in bf16 on the PE via the
      "reinterp" trick (rows on the contraction axis, no transpose needed).
      Host combines G across cores -> exact global BN1 stats -> folds BN1
      into W1,b1.
  L2: per-core main pass: PE-transpose x tiles, stacked-blockdiag MLP
      (32->16->1) on the PE, per-segment P = sum(x*a~), Q = sum(x),
      Sa = sum(a~), Sa2 = sum(a~^2) accumulated in PSUM.
      Host applies BN2 as an affine post-correction:
      seg_sum = s2*P + (s2*(b2-m2)+beta2)*Q, then mean + L2 normalize.
"""

import numpy as np
import ml_dtypes

import concourse.bass as bass
import concourse.tile as tile
from concourse import bacc, mybir
from concourse.bass_utils import run_bass_kernel_spmd

BF = ml_dtypes.bfloat16
F32 = mybir.dt.float32
BF16 = mybir.dt.bfloat16

NCORES = 8
PTS = 50000          # points per segment
SEGS_PER_CORE = 2
R = PTS * SEGS_PER_CORE   # rows per core
CIN = 32
CH = 16
N_TOTAL = NCORES * R
EPS_BN = 1e-5
EPS_NORM = 1e-12

PR_SEG = PTS // 16         # 3125 partition-rows per segment (16 rows each)
CHUNK_PR = 128             # partition-rows per full chunk
SEG_CHUNKS = [(t * CHUNK_PR, min(CHUNK_PR, PR_SEG - t * CHUNK_PR))
              for t in range((PR_SEG + CHUNK_PR - 1) // CHUNK_PR)]  # 24x128 + 53


def _build_gram():
    nc = bacc.Bacc("TRN2", target_bir_lowering=False, debug=False,
                   num_devices=NCORES)
    xb = nc.dram_tensor("xb", [R // 16, 512], BF16, kind="ExternalInput").ap()
    ones = nc.dram_tensor("ones", [128, 1], BF16, kind="ExternalInput").ap()
    oG = nc.dram_tensor("oG", [128, 129], F32, kind="ExternalOutput").ap()

    n_pr = R // 16  # 6250
    chunks = [(t * CHUNK_PR, min(CHUNK_PR, n_pr - t * CHUNK_PR))
              for t in range((n_pr + CHUNK_PR - 1) // CHUNK_PR)]

    with tile.TileContext(nc) as tc:
        with (
            tc.tile_pool(name="xin", bufs=4) as xin_pool,
            tc.tile_pool(name="consts", bufs=1) as cpool,
            tc.tile_pool(name="accp", bufs=1, space="PSUM") as acc_pool,
            tc.tile_pool(name="outs", bufs=1) as out_pool,
        ):
            ones_t = cpool.tile([128, 1], BF16)
            nc.sync.dma_start(ones_t[:], ones[:])
            acc = acc_pool.tile([128, 129], F32)  # G | S
            first = True
            for base, part in chunks:
                xt = xin_pool.tile([128, 512], BF16, tag="x")
                nc.sync.dma_start(xt[0:part, :], xb[base:base + part, :])
                for j in range(4):
                    sl = xt[0:part, 128 * j:128 * j + 128]
                    nc.tensor.matmul(acc[:, 0:128], sl, sl,
                                     start=first, stop=False)
                    first = False
                    nc.tensor.matmul(acc[:, 128:129], sl, ones_t[0:part, :],
                                     start=False, stop=False)
            outt = out_pool.tile([128, 129], F32)
            nc.scalar.copy(outt[:], acc[:])
            nc.sync.dma_start(oG[:], outt[:])
    nc.compile()
    return nc


def _build_main():
    nc = bacc.Bacc("TRN2", target_bir_lowering=False, debug=False,
                   num_devices=NCORES)
    xb = nc.dram_tensor("xb", [R // 16, 512], BF16, kind="ExternalInput").ap()
    W1s = nc.dram_tensor("W1s", [128, 64], BF16, kind="ExternalInput").ap()
    b1v = nc.dram_tensor("b1v", [64, 1], F32, kind="ExternalInput").ap()
    W2s = nc.dram_tensor("W2s", [64, 4], BF16, kind="ExternalInput").ap()
    ident = nc.dram_tensor("ident", [128, 128], BF16, kind="ExternalInput").ap()
    ident4 = nc.dram_tensor("ident4", [4, 4], BF16, kind="ExternalInput").ap()
    ones = nc.dram_tensor("ones", [128, 1], BF16, kind="ExternalInput").ap()
    zeros = nc.dram_tensor("zeros", [1, 512], BF16, kind="ExternalInput").ap()
    oACC = nc.dram_tensor("oACC", [128, 512], F32, kind="ExternalOutput").ap()

    with tile.TileContext(nc) as tc:
        with (
            tc.tile_pool(name="consts", bufs=1) as cpool,
            tc.tile_pool(name="xin", bufs=3) as xin_pool,
            tc.tile_pool(name="xtp", bufs=2, space="PSUM") as xtp_pool,
            tc.tile_pool(name="xts", bufs=2) as xts_pool,
            tc.tile_pool(name="hp", bufs=2, space="PSUM") as hp_pool,
            tc.tile_pool(name="hs", bufs=2) as hs_pool,
            tc.tile_pool(name="ap", bufs=1, space="PSUM") as apsum_pool,
            tc.tile_pool(name="as_", bufs=2) as as_pool,
            tc.tile_pool(name="atp", bufs=1, space="PSUM") as atp_pool,
            tc.tile_pool(name="ats", bufs=2) as ats_pool,
            tc.tile_pool(name="accp", bufs=1, space="PSUM") as acc_pool,
            tc.tile_pool(name="outs", bufs=1) as out_pool,
        ):
            w1_t = cpool.tile([128, 64], BF16)
            nc.sync.dma_start(w1_t[:], W1s[:])
            b1_t = cpool.tile([64, 1], F32)
            nc.sync.dma_start(b1_t[:], b1v[:])
            w2_t = cpool.tile([64, 4], BF16)
            nc.sync.dma_start(w2_t[:], W2s[:])
            id_t = cpool.tile([128, 128], BF16)
            nc.sync.dma_start(id_t[:], ident[:])
            id4_t = cpool.tile([4, 4], BF16)
            nc.sync.dma_start(id4_t[:], ident4[:])
            ones_t = cpool.tile([128, 1], BF16)
            nc.sync.dma_start(ones_t[:], ones[:])
            z_t = cpool.tile([1, 512], BF16)
            nc.sync.dma_start(z_t[:], zeros[:])

            acc = acc_pool.tile([128, 512], F32)
            # open one accumulation group covering the whole bank
            nc.tensor.matmul(acc[:, :], z_t[:, 0:128], z_t[:, :],
                             start=True, stop=False)

            for seg in range(SEGS_PER_CORE):
                off = 32 * seg
                seg_pr = seg * PR_SEG
                for base, part in SEG_CHUNKS:
                    xt = xin_pool.tile([128, 512], BF16, tag="x")
                    nc.sync.dma_start(
                        xt[0:part, :], xb[seg_pr + base: seg_pr + base + part, :])
                    # transpose x slices: XtP[32w4+c, 128*j+p] (col base 128j
                    # keeps PSUM writes 4B-aligned even when part=53)
                    xtp = xtp_pool.tile([128, 512], BF16, tag="xtp")
                    for j in range(4):
                        nc.tensor.transpose(
                            xtp[:, 128 * j: 128 * j + part],
                            xt[0:part, 128 * j: 128 * j + 128],
                            id_t[0:part, 0:part])
                    xts = xts_pool.tile([128, 512], BF16, tag="xts")
                    hp = hp_pool.tile([64, 512], F32, tag="h")
                    hs = hs_pool.tile([64, 512], BF16, tag="hr")
                    aps = apsum_pool.tile([4, 512], F32, tag="a")
                    as_t = as_pool.tile([4, 512], BF16, tag="as")
                    if part == 128:
                        spans = [(0, 512)]
                    else:
                        spans = [(128 * j, 128 * j + part) for j in range(4)]
                    for lo, hi in spans:
                        nc.scalar.copy(xts[:, lo:hi], xtp[:, lo:hi])
                        nc.tensor.matmul(hp[:, lo:hi], w1_t[:], xts[:, lo:hi],
                                         start=True, stop=True)
                        nc.scalar.activation(hs[:, lo:hi], hp[:, lo:hi],
                                             mybir.ActivationFunctionType.Relu,
                                             bias=b1_t[:])
                        nc.tensor.matmul(aps[:, lo:hi], w2_t[:], hs[:, lo:hi],
                                         start=True, stop=True)
                        nc.vector.tensor_copy(as_t[:, lo:hi], aps[:, lo:hi])
                    # transpose A back: At[p, 4j+d]
                    atp = atp_pool.tile([128, 16], BF16, tag="atp")
                    for j in range(4):
                        nc.tensor.transpose(
                            atp[0:part, 4 * j: 4 * j + 4],
                            as_t[:, 128 * j: 128 * j + part],
                            id4_t[:])
                    ats = ats_pool.tile([128, 16], BF16, tag="ats")
                    nc.vector.tensor_copy(ats[0:part, :], atp[0:part, :])
                    at2 = ats_pool.tile([128, 16], BF16, tag="at2")
                    nc.vector.tensor_mul(at2[0:part, :], ats[0:part, :],
                                         ats[0:part, :])
                    # P/Q/Sa/Sa2 accumulate
                    for j in range(4):
                        nc.tensor.matmul(
                            acc[off:off + 4, 0:128],
                            ats[0:part, 4 * j:4 * j + 4],
                            xt[0:part, 128 * j:128 * j + 128],
                            start=False, stop=False, tile_position=(0, off))
                    nc.tensor.matmul(acc[64 + off:65 + off, 0:512],
                                     ones_t[0:part, :], xt[0:part, :],
                                     start=False, stop=False,
                                     tile_position=(0, 64 + off))
                    nc.tensor.matmul(acc[off:off + 1, 384:400],
                                     ones_t[0:part, :], ats[0:part, :],
                                     start=False, stop=False,
                                     tile_position=(0, off))
                    nc.tensor.matmul(acc[off:off + 1, 400:416],
                                     ones_t[0:part, :], at2[0:part, :],
                                     start=False, stop=False,
                                     tile_position=(0, off))
            outt = out_pool.tile([128, 512], F32)
            nc.scalar.copy(outt[:], acc[:])
            nc.sync.dma_start(oACC[:], outt[:])
    nc.compile()
    return nc


_NC_CACHE = {}


def _get_nc(name):
    if name not in _NC_CACHE:
        _NC_CACHE[name] = _build_gram() if name == "gram" else _build_main()
    return _NC_CACHE[name]


def kernel(**inputs):
    x = np.asarray(inputs["x"], np.float32)
    W1 = np.asarray(inputs["W1"], np.float32)
    b1 = np.asarray(inputs["b1"], np.float64)
    g1 = np.asarray(inputs["gamma1"], np.float64)
    be1 = np.asarray(inputs["beta1"], np.float64)
    W2 = np.asarray(inputs["W2"], np.float32)
    b2 = np.asarray(inputs["b2"], np.float64)
    g2 = np.asarray(inputs["gamma2"], np.float64)
    be2 = np.asarray(inputs["beta2"], np.float64)
    length = np.asarray(inputs["length"], np.float32)

    N = x.shape[0]
    assert N == N_TOTAL
    xb = x.astype(BF)
    xb_cores = np.ascontiguousarray(xb.reshape(NCORES, R // 16, 512))

    ones_np = np.ones((128, 1), BF)
    core_ids = list(range(NCORES))

    # ---- launch 1: Gram ----
    nc1 = _get_nc("gram")
    in_maps1 = [{"xb": xb_cores[k], "ones": ones_np} for k in core_ids]
    res1 = run_bass_kernel_spmd(nc1, in_maps1, core_ids).results
    G = np.zeros((128, 129), np.float64)
    for k in core_ids:
        G += res1[k]["oG"]
    xtx = np.zeros((32, 32), np.float64)
    sx = np.zeros(32, np.float64)
    for d in range(4):
        xtx += G[32 * d:32 * d + 32, 32 * d:32 * d + 32]
        sx += G[32 * d:32 * d + 32, 128]
    mean = sx / N
    C = xtx / N - np.outer(mean, mean)
    W1d = W1.astype(np.float64)
    var_h = np.einsum('jc,cd,jd->j', W1d, C, W1d)
    m_h = W1d @ mean + b1
    s1 = g1 / np.sqrt(var_h + EPS_BN)
    W1fold = (W1d * s1[:, None])
    b1fold = (s1 * (b1 - m_h) + be1)

    W1s_np = np.zeros((128, 64), np.float32)
    b1v_np = np.zeros((64, 1), np.float32)
    W2s_np = np.zeros((64, 4), np.float32)
    for w4 in range(4):
        W1s_np[32 * w4:32 * w4 + 32, 16 * w4:16 * w4 + 16] = W1fold.T
        b1v_np[16 * w4:16 * w4 + 16, 0] = b1fold
        W2s_np[16 * w4:16 * w4 + 16, w4] = W2[0]
    ident_np = np.eye(128, dtype=BF)
    ident4_np = np.eye(4, dtype=BF)
    zeros_np = np.zeros((1, 512), BF)

    # ---- launch 2: main ----
    nc2 = _get_nc("main")
    common = {"W1s": W1s_np.astype(BF), "b1v": b1v_np,
              "W2s": W2s_np.astype(BF), "ident": ident_np,
              "ident4": ident4_np, "ones": ones_np, "zeros": zeros_np}
    in_maps2 = [{"xb": xb_cores[k], **common} for k in core_ids]
    res2 = run_bass_kernel_spmd(nc2, in_maps2, core_ids).results

    P = np.zeros((16, 32), np.float64)
    Q = np.zeros((16, 32), np.float64)
    Sa = 0.0
    Sa2 = 0.0
    for k in core_ids:
        ACC = res2[k]["oACC"].astype(np.float64)
        for seg in range(SEGS_PER_CORE):
            off = 32 * seg
            s = SEGS_PER_CORE * k + seg
            for d in range(4):
                P[s] += ACC[off + d, 32 * d:32 * d + 32]
            Q[s] += ACC[64 + off, 0:512].reshape(16, 32).sum(axis=0)
            Sa += ACC[off, 384:400].sum()
            Sa2 += ACC[off, 400:416].sum()

    b2d = float(b2[0])
    m2 = (Sa + N * b2d) / N
    e2 = (Sa2 + 2 * b2d * Sa + N * b2d * b2d) / N
    v2 = e2 - m2 * m2
    s2 = float(g2[0]) / np.sqrt(v2 + EPS_BN)
    seg_sum = s2 * P + (s2 * (b2d - m2) + float(be2[0])) * Q
    result = seg_sum / length.astype(np.float64)[:, None]
    norm = np.linalg.norm(result, axis=1, keepdims=True)
    out = result / np.maximum(norm, EPS_NORM)
    return out.astype(np.float32)


# revision 7
# speedup vs baseline: 9000.7188x; 9000.7188x over previous
"""Trainium2 Bass kernel for nn_FCGF_point_att3 (segment_reduce).

Pipeline (per reference.py):
  h = x@W1.T + b1 ; h = relu(BN(h)) ; a = BN(h@W2.T + b2)
  out = l2norm(segment_mean(x * a))   with global (all-N) BN stats.

Strategy: 8-way data parallel over segments (2 segments of 50k points per
core).  Two SPMD launches:
  L1: per-core Gram matrix G = [X|1]^T[X|1] in bf16 on the PE via the
      "reinterp" trick (rows on the contraction axis, no transpose needed).
      Host combines G across cores -> exact global BN1 stats -> folds BN1
      into W1,b1.
  L2: per-core main pass: PE-transpose x tiles, stacked-blockdiag MLP
      (32->16->1) on the PE, per-segment P = sum(x*a~), Q = sum(x),
      Sa = sum(a~), Sa2 = sum(a~^2) accumulated in PSUM.
      Host applies BN2 as an affine post-correction:
      seg_sum = s2*P + (s2*(b2-m2)+beta2)*Q, then mean + L2 normalize.
"""

import numpy as np
import ml_dtypes

import concourse.bass as bass
import concourse.tile as tile
from concourse import bacc, mybir
from concourse.bass_utils import run_bass_kernel_spmd

BF = ml_dtypes.bfloat16
F32 = mybir.dt.float32
BF16 = mybir.dt.bfloat16

NCORES = 8
PTS = 50000          # points per segment
SEGS_PER_CORE = 2
R = PTS * SEGS_PER_CORE   # rows per core
CIN = 32
CH = 16
N_TOTAL = NCORES * R
EPS_BN = 1e-5
EPS_NORM = 1e-12

PR_SEG = PTS // 16         # 3125 partition-rows per segment (16 rows each)
CHUNK_PR = 128             # partition-rows per full chunk
SEG_CHUNKS = [(t * CHUNK_PR, min(CHUNK_PR, PR_SEG - t * CHUNK_PR))
              for t in range((PR_SEG + CHUNK_PR - 1) // CHUNK_PR)]  # 24x128 + 53


def _build_gram():
    nc = bacc.Bacc("TRN2", target_bir_lowering=False, debug=False,
                   num_devices=NCORES)
    xb = nc.dram_tensor("xb", [R // 16, 512], BF16, kind="ExternalInput").ap()
    ones = nc.dram_tensor("ones", [128, 1], BF16, kind="ExternalInput").ap()
    oG = nc.dram_tensor("oG", [128, 129], F32, kind="ExternalOutput").ap()

    n_pr = R // 16  # 6250
    chunks = [(t * CHUNK_PR, min(CHUNK_PR, n_pr - t * CHUNK_PR))
              for t in range((n_pr + CHUNK_PR - 1) // CHUNK_PR)]

    with tile.TileContext(nc) as tc:
        with (
            tc.tile_pool(name="xin", bufs=4) as xin_pool,
            tc.tile_pool(name="consts", bufs=1) as cpool,
            tc.tile_pool(name="accp", bufs=1, space="PSUM") as acc_pool,
            tc.tile_pool(name="outs", bufs=1) as out_pool,
        ):
            ones_t = cpool.tile([128, 1], BF16)
            nc.sync.dma_start(ones_t[:], ones[:])
            acc = acc_pool.tile([128, 129], F32)  # G | S
            first = True
            for base, part in chunks:
                xt = xin_pool.tile([128, 512], BF16, tag="x")
                nc.sync.dma_start(xt[0:part, :], xb[base:base + part, :])
                for j in range(4):
                    sl = xt[0:part, 128 * j:128 * j + 128]
                    nc.tensor.matmul(acc[:, 0:128], sl, sl,
                                     start=first, stop=False)
                    first = False
                    nc.tensor.matmul(acc[:, 128:129], sl, ones_t[0:part, :],
                                     start=False, stop=False)
            outt = out_pool.tile([128, 129], F32)
            nc.scalar.copy(outt[:], acc[:])
            nc.sync.dma_start(oG[:], outt[:])
    nc.compile()
    return nc


def _build_main():
    nc = bacc.Bacc("TRN2", target_bir_lowering=False, debug=False,
                   num_devices=NCORES)
    xb = nc.dram_tensor("xb", [R // 16, 512], BF16, kind="ExternalInput").ap()
    W1s = nc.dram_tensor("W1s", [128, 64], BF16, kind="ExternalInput").ap()
    b1v = nc.dram_tensor("b1v", [64, 1], F32, kind="ExternalInput").ap()
    W2s = nc.dram_tensor("W2s", [64, 4], BF16, kind="ExternalInput").ap()
    ident = nc.dram_tensor("ident", [128, 128], BF16, kind="ExternalInput").ap()
    ident4 = nc.dram_tensor("ident4", [4, 4], BF16, kind="ExternalInput").ap()
    ones = nc.dram_tensor("ones", [128, 1], BF16, kind="ExternalInput").ap()
    zeros = nc.dram_tensor("zeros", [1, 512], BF16, kind="ExternalInput").ap()
    oACC = nc.dram_tensor("oACC", [128, 512], F32, kind="ExternalOutput").ap()

    with tile.TileContext(nc) as tc:
        with (
            tc.tile_pool(name="consts", bufs=1) as cpool,
            tc.tile_pool(name="xin", bufs=3) as xin_pool,
            tc.tile_pool(name="xtp", bufs=2, space="PSUM") as xtp_pool,
            tc.tile_pool(name="xts", bufs=2) as xts_pool,
            tc.tile_pool(name="hp", bufs=2, space="PSUM") as hp_pool,
            tc.tile_pool(name="hs", bufs=2) as hs_pool,
            tc.tile_pool(name="ap", bufs=1, space="PSUM") as apsum_pool,
            tc.tile_pool(name="as_", bufs=2) as as_pool,
            tc.tile_pool(name="atp", bufs=1, space="PSUM") as atp_pool,
            tc.tile_pool(name="ats", bufs=2) as ats_pool,
            tc.tile_pool(name="accp", bufs=1, space="PSUM") as acc_pool,
            tc.tile_pool(name="outs", bufs=1) as out_pool,
        ):
            w1_t = cpool.tile([128, 64], BF16)
            nc.sync.dma_start(w1_t[:], W1s[:])
            b1_t = cpool.tile([64, 1], F32)
            nc.sync.dma_start(b1_t[:], b1v[:])
            w2_t = cpool.tile([64, 4], BF16)
            nc.sync.dma_start(w2_t[:], W2s[:])
            id_t = cpool.tile([128, 128], BF16)
            nc.sync.dma_start(id_t[:], ident[:])
            id4_t = cpool.tile([4, 4], BF16)
            nc.sync.dma_start(id4_t[:], ident4[:])
            ones_t = cpool.tile([128, 1], BF16)
            nc.sync.dma_start(ones_t[:], ones[:])
            z_t = cpool.tile([1, 512], BF16)
            nc.sync.dma_start(z_t[:], zeros[:])

            acc = acc_pool.tile([128, 512], F32)
            # open one accumulation group covering the whole bank
            nc.tensor.matmul(acc[:, :], z_t[:, 0:128], z_t[:, :],
                             start=True, stop=False)

            for seg in range(SEGS_PER_CORE):
                off = 32 * seg
                seg_pr = seg * PR_SEG
                for base, part in SEG_CHUNKS:
                    xt = xin_pool.tile([128, 512], BF16, tag="x")
                    nc.sync.dma_start(
                        xt[0:part, :], xb[seg_pr + base: seg_pr + base + part, :])
                    # transpose x slices: XtP[32w4+c, 128*j+p] (col base 128j
                    # keeps PSUM writes 4B-aligned even when part=53)
                    xtp = xtp_pool.tile([128, 512], BF16, tag="xtp")
                    for j in range(4):
                        nc.tensor.transpose(
                            xtp[:, 128 * j: 128 * j + part],
                            xt[0:part, 128 * j: 128 * j + 128],
                            id_t[0:part, 0:part])
                    xts = xts_pool.tile([128, 512], BF16, tag="xts")
                    hp = hp_pool.tile([64, 512], F32, tag="h")
                    hs = hs_pool.tile([64, 512], BF16, tag="hr")
                    aps = apsum_pool.tile([4, 512], F32, tag="a")
                    as_t = as_pool.tile([4, 512], BF16, tag="as")
                    if part == 128:
                        spans = [(0, 512)]
                    else:
                        spans = [(128 * j, 128 * j + part) for j in range(4)]
                    for lo, hi in spans:
                        nc.scalar.copy(xts[:, lo:hi], xtp[:, lo:hi])
                        nc.tensor.matmul(hp[:, lo:hi], w1_t[:], xts[:, lo:hi],
                                         start=True, stop=True)
                        nc.scalar.activation(hs[:, lo:hi], hp[:, lo:hi],
                                             mybir.ActivationFunctionType.Relu,
                                             bias=b1_t[:])
                        nc.tensor.matmul(aps[:, lo:hi], w2_t[:], hs[:, lo:hi],
                                         start=True, stop=True)
                        nc.vector.tensor_copy(as_t[:, lo:hi], aps[:, lo:hi])
                    # transpose A back: At[p, 4j+d]
                    atp = atp_pool.tile([128, 16], BF16, tag="atp")
                    for j in range(4):
                        nc.tensor.transpose(
                            atp[0:part, 4 * j: 4 * j + 4],
                            as_t[:, 128 * j: 128 * j + part],
                            id4_t[:])
                    ats = ats_pool.tile([128, 16], BF16, tag="ats")
                    nc.vector.tensor_copy(ats[0:part, :], atp[0:part, :])
                    at2 = ats_pool.tile([128, 16], BF16, tag="at2")
                    nc.vector.tensor_mul(at2[0:part, :], ats[0:part, :],
                                         ats[0:part, :])
                    # P/Q/Sa/Sa2 accumulate
                    for j in range(4):
                        nc.tensor.matmul(
                            acc[off:off + 4, 0:128],
                            ats[0:part, 4 * j:4 * j + 4],
                            xt[0:part, 128 * j:128 * j + 128],
                            start=False, stop=False, tile_position=(0, off))
                    nc.tensor.matmul(acc[64 + off:65 + off, 0:512],
                                     ones_t[0:part, :], xt[0:part, :],
                                     start=False, stop=False,
                                     tile_position=(0, 64 + off))
                    nc.tensor.matmul(acc[off:off + 1, 384:400],
                                     ones_t[0:part, :], ats[0:part, :],
                                     start=False, stop=False,
                                     tile_position=(0, off))
                    nc.tensor.matmul(acc[off:off + 1, 400:416],
                                     ones_t[0:part, :], at2[0:part, :],
                                     start=False, stop=False,
                                     tile_position=(0, off))
            outt = out_pool.tile([128, 512], F32)
            nc.scalar.copy(outt[:], acc[:])
            nc.sync.dma_start(oACC[:], outt[:])
    nc.compile()
    return nc


QCHUNK = 512          # quads per full main-pass chunk (2048 rows)
SEG_Q = PTS // 4      # 12500 quads per segment
FULL_CHUNKS = 24      # 24*512 quads; tail = 212 quads = 848 rows (53 pr)


def _build_main2():
    """v2: DMA-transposed quad-view main pass; v1-style PE-transpose tail."""
    nc = bacc.Bacc("TRN2", target_bir_lowering=False, debug=False,
                   num_devices=NCORES)
    xb = nc.dram_tensor("xb", [R // 4, 128], BF16, kind="ExternalInput").ap()
    W1s = nc.dram_tensor("W1s", [128, 64], BF16, kind="ExternalInput").ap()
    b1v = nc.dram_tensor("b1v", [64, 1], F32, kind="ExternalInput").ap()
    W2s = nc.dram_tensor("W2s", [64, 4], BF16, kind="ExternalInput").ap()
    ident = nc.dram_tensor("ident", [128, 128], BF16, kind="ExternalInput").ap()
    ident4 = nc.dram_tensor("ident4", [4, 4], BF16, kind="ExternalInput").ap()
    ones = nc.dram_tensor("ones", [128, 1], BF16, kind="ExternalInput").ap()
    zeros = nc.dram_tensor("zeros", [1, 512], BF16, kind="ExternalInput").ap()
    oACC = nc.dram_tensor("oACC", [128, 512], F32, kind="ExternalOutput").ap()
    oACC2 = nc.dram_tensor("oACC2", [128, 512], F32, kind="ExternalOutput").ap()

    xb16 = xb.rearrange("(p k) c -> p (k c)", k=4)  # [R//16, 512] natural view

    with tile.TileContext(nc) as tc:
        with (
            tc.tile_pool(name="consts", bufs=1) as cpool,
            tc.tile_pool(name="xT", bufs=3) as xT_pool,
            tc.tile_pool(name="xq", bufs=3) as xq_pool,
            tc.tile_pool(name="hp", bufs=2, space="PSUM") as hp_pool,
            tc.tile_pool(name="hs", bufs=2) as hs_pool,
            tc.tile_pool(name="ap", bufs=2, space="PSUM") as apsum_pool,
            tc.tile_pool(name="as_", bufs=2) as as_pool,
            tc.tile_pool(name="atp", bufs=1, space="PSUM") as atp_pool,
            tc.tile_pool(name="ats", bufs=2) as ats_pool,
            tc.tile_pool(name="xtp", bufs=1, space="PSUM") as xtp_pool,
            tc.tile_pool(name="acc", bufs=1, space="PSUM") as acc_pool,
            tc.tile_pool(name="outs", bufs=1) as out_pool,
        ):
            w1_t = cpool.tile([128, 64], BF16)
            nc.sync.dma_start(w1_t[:], W1s[:])
            b1_t = cpool.tile([64, 1], F32)
            nc.sync.dma_start(b1_t[:], b1v[:])
            w2_t = cpool.tile([64, 4], BF16)
            nc.sync.dma_start(w2_t[:], W2s[:])
            id_t = cpool.tile([128, 128], BF16)
            nc.sync.dma_start(id_t[:], ident[:])
            id4_t = cpool.tile([4, 4], BF16)
            nc.sync.dma_start(id4_t[:], ident4[:])
            ones_t = cpool.tile([128, 1], BF16)
            nc.sync.dma_start(ones_t[:], ones[:])
            z_t = cpool.tile([1, 512], BF16)
            nc.sync.dma_start(z_t[:], zeros[:])

            acc = acc_pool.tile([128, 512], F32, tag="acc")
            acc2 = acc_pool.tile([128, 512], F32, tag="acc2")
            nc.tensor.matmul(acc[:, :], z_t[:, 0:128], z_t[:, :],
                             start=True, stop=False)
            nc.tensor.matmul(acc2[:, :], z_t[:, 0:128], z_t[:, :],
                             start=True, stop=False)

            def mlp(xts_ap, part4, hp, hs, aps, as_t):
                """xts_ap: [128, part4] transposed input (SBUF bf16)."""
                nc.tensor.matmul(hp[:, 0:part4], w1_t[:], xts_ap,
                                 start=True, stop=True)
                nc.scalar.activation(hs[:, 0:part4], hp[:, 0:part4],
                                     mybir.ActivationFunctionType.Relu,
                                     bias=b1_t[:])
                nc.tensor.matmul(aps[:, 0:part4], w2_t[:], hs[:, 0:part4],
                                 start=True, stop=True)
                nc.vector.tensor_copy(as_t[:, 0:part4], aps[:, 0:part4])

            for seg in range(SEGS_PER_CORE):
                off = 32 * seg
                seg_q0 = seg * SEG_Q
                for t in range(FULL_CHUNKS):
                    q0 = seg_q0 + t * QCHUNK
                    pr0 = q0 // 4  # natural 16-row partition offset
                    xT = xT_pool.tile([128, 512], BF16, tag="xT")
                    nc.sync.dma_start(xT[:], xb[q0:q0 + QCHUNK, :],
                                      transpose=True)
                    xq = xq_pool.tile([128, 512], BF16, tag="xq")
                    nc.sync.dma_start(xq[:], xb16[pr0:pr0 + 128, :])
                    hp = hp_pool.tile([64, 512], F32, tag="h")
                    hs = hs_pool.tile([64, 512], BF16, tag="hr")
                    aps = apsum_pool.tile([4, 512], F32, tag="a")
                    as_t = as_pool.tile([4, 512], BF16, tag="as")
                    mlp(xT[:, :], 512, hp, hs, aps, as_t)
                    # bridge quad-order A -> 16-row natural order via
                    # stride-4 column slices: At[p, 4j+d] = As[d, 4p+j]
                    as3 = as_t[:, :].rearrange("g (p j) -> g j p", j=4)
                    atp = atp_pool.tile([128, 16], BF16, tag="atp")
                    for j in range(4):
                        nc.tensor.transpose(
                            atp[:, 4 * j: 4 * j + 4],
                            as3[:, j, :],
                            id4_t[:])
                    ats = ats_pool.tile([128, 16], BF16, tag="ats")
                    nc.vector.tensor_copy(ats[:, :], atp[:, :])
                    at2 = ats_pool.tile([128, 16], BF16, tag="at2")
                    nc.vector.tensor_mul(at2[:, :], ats[:, :], ats[:, :])
                    nc.tensor.matmul(acc[off:off + 16, 0:512], ats[:, :],
                                     xq[:, :], start=False, stop=False,
                                     tile_position=(0, off))
                    nc.tensor.matmul(acc[64 + off:65 + off, 0:512],
                                     ones_t[:, :], xq[:, :],
                                     start=False, stop=False,
                                     tile_position=(0, 64 + off))
                    nc.tensor.matmul(acc2[off:off + 1, 0:16],
                                     ones_t[:, :], ats[:, :],
                                     start=False, stop=False,
                                     tile_position=(0, off))
                    nc.tensor.matmul(acc2[off:off + 1, 16:32],
                                     ones_t[:, :], at2[:, :],
                                     start=False, stop=False,
                                     tile_position=(0, off))
                # ---- tail: 848 rows via v1 PE-transpose path ----
                part = 53
                pr0 = (seg * PTS + FULL_CHUNKS * QCHUNK * 4) // 16
                xt = xq_pool.tile([128, 512], BF16, tag="xq")
                nc.sync.dma_start(xt[0:part, :], xb16[pr0:pr0 + part, :])
                xtp = xtp_pool.tile([128, 512], BF16, tag="xtp")
                for j in range(4):
                    nc.tensor.transpose(
                        xtp[:, 128 * j: 128 * j + part],
                        xt[0:part, 128 * j: 128 * j + 128],
                        id_t[0:part, 0:part])
                xts = xT_pool.tile([128, 512], BF16, tag="xT")
                hp = hp_pool.tile([64, 512], F32, tag="h")
                hs = hs_pool.tile([64, 512], BF16, tag="hr")
                aps = apsum_pool.tile([4, 512], F32, tag="a")
                as_t = as_pool.tile([4, 512], BF16, tag="as")
                for j in range(4):
                    lo, hi = 128 * j, 128 * j + part
                    nc.scalar.copy(xts[:, lo:hi], xtp[:, lo:hi])
                    nc.tensor.matmul(hp[:, lo:hi], w1_t[:], xts[:, lo:hi],
                                     start=True, stop=True)
                    nc.scalar.activation(hs[:, lo:hi], hp[:, lo:hi],
                                         mybir.ActivationFunctionType.Relu,
                                         bias=b1_t[:])
                    nc.tensor.matmul(aps[:, lo:hi], w2_t[:], hs[:, lo:hi],
                                     start=True, stop=True)
                    nc.vector.tensor_copy(as_t[:, lo:hi], aps[:, lo:hi])
                atp = atp_pool.tile([128, 16], BF16, tag="atp")
                for j in range(4):
                    nc.tensor.transpose(
                        atp[0:part, 4 * j: 4 * j + 4],
                        as_t[:, 128 * j: 128 * j + part],
                        id4_t[:])
                ats = ats_pool.tile([128, 16], BF16, tag="ats")
                nc.vector.tensor_copy(ats[0:part, :], atp[0:part, :])
                at2 = ats_pool.tile([128, 16], BF16, tag="at2")
                nc.vector.tensor_mul(at2[0:part, :], ats[0:part, :],
                                     ats[0:part, :])
                for j in range(4):
                    nc.tensor.matmul(
                        acc2[off:off + 4, 32:160],
                        ats[0:part, 4 * j:4 * j + 4],
                        xt[0:part, 128 * j:128 * j + 128],
                        start=False, stop=False, tile_position=(0, off))
                nc.tensor.matmul(acc[64 + off:65 + off, 0:512],
                                 ones_t[0:part, :], xt[0:part, :],
                                 start=False, stop=False,
                                 tile_position=(0, 64 + off))
                nc.tensor.matmul(acc2[off:off + 1, 0:16],
                                 ones_t[0:part, :], ats[0:part, :],
                                 start=False, stop=False,
                                 tile_position=(0, off))
                nc.tensor.matmul(acc2[off:off + 1, 16:32],
                                 ones_t[0:part, :], at2[0:part, :],
                                 start=False, stop=False,
                                 tile_position=(0, off))
            outt = out_pool.tile([128, 512], F32, tag="o1")
            nc.scalar.copy(outt[:], acc[:])
            nc.sync.dma_start(oACC[:], outt[:])
            outt2 = out_pool.tile([128, 512], F32, tag="o2")
            nc.scalar.copy(outt2[:], acc2[:])
            nc.sync.dma_start(oACC2[:], outt2[:])
    nc.compile()
    return nc


_NC_CACHE = {}


def _get_nc(name):
    if name not in _NC_CACHE:
        _NC_CACHE[name] = _build_gram() if name == "gram" else _build_main2()
    return _NC_CACHE[name]


def kernel(**inputs):
    x = np.asarray(inputs["x"], np.float32)
    W1 = np.asarray(inputs["W1"], np.float32)
    b1 = np.asarray(inputs["b1"], np.float64)
    g1 = np.asarray(inputs["gamma1"], np.float64)
    be1 = np.asarray(inputs["beta1"], np.float64)
    W2 = np.asarray(inputs["W2"], np.float32)
    b2 = np.asarray(inputs["b2"], np.float64)
    g2 = np.asarray(inputs["gamma2"], np.float64)
    be2 = np.asarray(inputs["beta2"], np.float64)
    length = np.asarray(inputs["length"], np.float32)

    N = x.shape[0]
    assert N == N_TOTAL
    xb = x.astype(BF)
    xb_cores = np.ascontiguousarray(xb.reshape(NCORES, R // 16, 512))

    ones_np = np.ones((128, 1), BF)
    core_ids = list(range(NCORES))

    # ---- launch 1: Gram ----
    nc1 = _get_nc("gram")
    in_maps1 = [{"xb": xb_cores[k], "ones": ones_np} for k in core_ids]
    res1 = run_bass_kernel_spmd(nc1, in_maps1, core_ids).results
    G = np.zeros((128, 129), np.float64)
    for k in core_ids:
        G += res1[k]["oG"]
    xtx = np.zeros((32, 32), np.float64)
    sx = np.zeros(32, np.float64)
    for d in range(4):
        xtx += G[32 * d:32 * d + 32, 32 * d:32 * d + 32]
        sx += G[32 * d:32 * d + 32, 128]
    mean = sx / N
    C = xtx / N - np.outer(mean, mean)
    W1d = W1.astype(np.float64)
    var_h = np.einsum('jc,cd,jd->j', W1d, C, W1d)
    m_h = W1d @ mean + b1
    s1 = g1 / np.sqrt(var_h + EPS_BN)
    W1fold = (W1d * s1[:, None])
    b1fold = (s1 * (b1 - m_h) + be1)

    W1s_np = np.zeros((128, 64), np.float32)
    b1v_np = np.zeros((64, 1), np.float32)
    W2s_np = np.zeros((64, 4), np.float32)
    for w4 in range(4):
        W1s_np[32 * w4:32 * w4 + 32, 16 * w4:16 * w4 + 16] = W1fold.T
        b1v_np[16 * w4:16 * w4 + 16, 0] = b1fold
        W2s_np[16 * w4:16 * w4 + 16, w4] = W2[0]
    ident_np = np.eye(128, dtype=BF)
    ident4_np = np.eye(4, dtype=BF)
    zeros_np = np.zeros((1, 512), BF)

    # ---- launch 2: main (v2 layout) ----
    nc2 = _get_nc("main")
    common = {"W1s": W1s_np.astype(BF), "b1v": b1v_np,
              "W2s": W2s_np.astype(BF), "ident": ident_np,
              "ident4": ident4_np, "ones": ones_np, "zeros": zeros_np}
    xb_cores4 = xb_cores.reshape(NCORES, R // 4, 128)
    in_maps2 = [{"xb": xb_cores4[k], **common} for k in core_ids]
    res2 = run_bass_kernel_spmd(nc2, in_maps2, core_ids).results

    P = np.zeros((16, 32), np.float64)
    Q = np.zeros((16, 32), np.float64)
    Sa = 0.0
    Sa2 = 0.0
    for k in core_ids:
        ACC = res2[k]["oACC"].astype(np.float64)
        ACC2 = res2[k]["oACC2"].astype(np.float64)
        for seg in range(SEGS_PER_CORE):
            off = 32 * seg
            s = SEGS_PER_CORE * k + seg
            for t in range(4):
                for g in range(4):
                    P[s] += ACC[off + 4 * t + g,
                                128 * t + 32 * g: 128 * t + 32 * g + 32]
            for d in range(4):
                P[s] += ACC2[off + d, 32 + 32 * d: 64 + 32 * d]
            Q[s] += ACC[64 + off, 0:512].reshape(16, 32).sum(axis=0)
            Sa += ACC2[off, 0:16].sum()
            Sa2 += ACC2[off, 16:32].sum()

    b2d = float(b2[0])
    m2 = (Sa + N * b2d) / N
    e2 = (Sa2 + 2 * b2d * Sa + N * b2d * b2d) / N
    v2 = e2 - m2 * m2
    s2 = float(g2[0]) / np.sqrt(v2 + EPS_BN)
    seg_sum = s2 * P + (s2 * (b2d - m2) + float(be2[0])) * Q
    result = seg_sum / length.astype(np.float64)[:, None]
    norm = np.linalg.norm(result, axis=1, keepdims=True)
    out = result / np.maximum(norm, EPS_NORM)
    return out.astype(np.float32)


# revision 9
# speedup vs baseline: 14580.2576x; 1.6199x over previous
"""Trainium2 Bass kernel for nn_FCGF_point_att3 (segment_reduce).

Pipeline (per reference.py):
  h = x@W1.T + b1 ; h = relu(BN(h)) ; a = BN(h@W2.T + b2)
  out = l2norm(segment_mean(x * a))   with global (all-N) BN stats.

Strategy: 8-way data parallel over segments (2 segments of 50k points per
core).  Two SPMD launches:
  L1: per-core Gram matrix G = [X|1]^T[X|1] in bf16 on the PE via the
      "reinterp" trick (rows on the contraction axis, no transpose needed).
      Host combines G across cores -> exact global BN1 stats -> folds BN1
      into W1,b1.
  L2: per-core main pass: PE-transpose x tiles, stacked-blockdiag MLP
      (32->16->1) on the PE, per-segment P = sum(x*a~), Q = sum(x),
      Sa = sum(a~), Sa2 = sum(a~^2) accumulated in PSUM.
      Host applies BN2 as an affine post-correction:
      seg_sum = s2*P + (s2*(b2-m2)+beta2)*Q, then mean + L2 normalize.
"""

import numpy as np
import ml_dtypes

import concourse.bass as bass
import concourse.tile as tile
from concourse import bacc, mybir
from concourse.bass_utils import run_bass_kernel_spmd

BF = ml_dtypes.bfloat16
F32 = mybir.dt.float32
BF16 = mybir.dt.bfloat16

NCORES = 8
PTS = 50000          # points per segment
SEGS_PER_CORE = 2
R = PTS * SEGS_PER_CORE   # rows per core
CIN = 32
CH = 16
N_TOTAL = NCORES * R
EPS_BN = 1e-5
EPS_NORM = 1e-12

PR_SEG = PTS // 16         # 3125 partition-rows per segment (16 rows each)
CHUNK_PR = 128             # partition-rows per full chunk
SEG_CHUNKS = [(t * CHUNK_PR, min(CHUNK_PR, PR_SEG - t * CHUNK_PR))
              for t in range((PR_SEG + CHUNK_PR - 1) // CHUNK_PR)]  # 24x128 + 53


def _build_gram():
    nc = bacc.Bacc("TRN2", target_bir_lowering=False, debug=False,
                   num_devices=NCORES)
    xb = nc.dram_tensor("xb", [R // 16, 512], BF16, kind="ExternalInput").ap()
    ones = nc.dram_tensor("ones", [128, 1], BF16, kind="ExternalInput").ap()
    oG = nc.dram_tensor("oG", [128, 129], F32, kind="ExternalOutput").ap()

    # pair rows: [3125, 1024] view, 128-partition tiles hold 4096 rows each
    xb2 = xb.rearrange("(a b) c -> a (b c)", b=2)
    n_pr2 = R // 32  # 3125
    chunks = [(t * CHUNK_PR, min(CHUNK_PR, n_pr2 - t * CHUNK_PR))
              for t in range((n_pr2 + CHUNK_PR - 1) // CHUNK_PR)]

    with tile.TileContext(nc) as tc:
        with (
            tc.tile_pool(name="xin", bufs=4) as xin_pool,
            tc.tile_pool(name="consts", bufs=1) as cpool,
            tc.tile_pool(name="accp", bufs=1, space="PSUM") as acc_pool,
            tc.tile_pool(name="outs", bufs=1) as out_pool,
        ):
            ones_t = cpool.tile([128, 1], BF16)
            nc.sync.dma_start(ones_t[:], ones[:])
            acc = acc_pool.tile([128, 129], F32)  # G | S
            first = True
            for base, part in chunks:
                xt = xin_pool.tile([128, 1024], BF16, tag="x")
                nc.sync.dma_start(xt[0:part, :], xb2[base:base + part, :])
                for j in range(8):
                    sl = xt[0:part, 128 * j:128 * j + 128]
                    nc.tensor.matmul(acc[:, 0:128], sl, sl,
                                     start=first, stop=False)
                    first = False
                    nc.tensor.matmul(acc[:, 128:129], sl, ones_t[0:part, :],
                                     start=False, stop=False)
            outt = out_pool.tile([128, 129], F32)
            nc.scalar.copy(outt[:], acc[:])
            nc.sync.dma_start(oG[:], outt[:])
    nc.compile()
    return nc


def _build_main():
    nc = bacc.Bacc("TRN2", target_bir_lowering=False, debug=False,
                   num_devices=NCORES)
    xb = nc.dram_tensor("xb", [R // 16, 512], BF16, kind="ExternalInput").ap()
    W1s = nc.dram_tensor("W1s", [128, 64], BF16, kind="ExternalInput").ap()
    b1v = nc.dram_tensor("b1v", [64, 1], F32, kind="ExternalInput").ap()
    W2s = nc.dram_tensor("W2s", [64, 4], BF16, kind="ExternalInput").ap()
    ident = nc.dram_tensor("ident", [128, 128], BF16, kind="ExternalInput").ap()
    ident4 = nc.dram_tensor("ident4", [4, 4], BF16, kind="ExternalInput").ap()
    ones = nc.dram_tensor("ones", [128, 1], BF16, kind="ExternalInput").ap()
    zeros = nc.dram_tensor("zeros", [1, 512], BF16, kind="ExternalInput").ap()
    oACC = nc.dram_tensor("oACC", [128, 512], F32, kind="ExternalOutput").ap()

    with tile.TileContext(nc) as tc:
        with (
            tc.tile_pool(name="consts", bufs=1) as cpool,
            tc.tile_pool(name="xin", bufs=3) as xin_pool,
            tc.tile_pool(name="xtp", bufs=2, space="PSUM") as xtp_pool,
            tc.tile_pool(name="xts", bufs=2) as xts_pool,
            tc.tile_pool(name="hp", bufs=2, space="PSUM") as hp_pool,
            tc.tile_pool(name="hs", bufs=2) as hs_pool,
            tc.tile_pool(name="ap", bufs=1, space="PSUM") as apsum_pool,
            tc.tile_pool(name="as_", bufs=2) as as_pool,
            tc.tile_pool(name="atp", bufs=1, space="PSUM") as atp_pool,
            tc.tile_pool(name="ats", bufs=2) as ats_pool,
            tc.tile_pool(name="accp", bufs=1, space="PSUM") as acc_pool,
            tc.tile_pool(name="outs", bufs=1) as out_pool,
        ):
            w1_t = cpool.tile([128, 64], BF16)
            nc.sync.dma_start(w1_t[:], W1s[:])
            b1_t = cpool.tile([64, 1], F32)
            nc.sync.dma_start(b1_t[:], b1v[:])
            w2_t = cpool.tile([64, 4], BF16)
            nc.sync.dma_start(w2_t[:], W2s[:])
            id_t = cpool.tile([128, 128], BF16)
            nc.sync.dma_start(id_t[:], ident[:])
            id4_t = cpool.tile([4, 4], BF16)
            nc.sync.dma_start(id4_t[:], ident4[:])
            ones_t = cpool.tile([128, 1], BF16)
            nc.sync.dma_start(ones_t[:], ones[:])
            z_t = cpool.tile([1, 512], BF16)
            nc.sync.dma_start(z_t[:], zeros[:])

            acc = acc_pool.tile([128, 512], F32)
            # open one accumulation group covering the whole bank
            nc.tensor.matmul(acc[:, :], z_t[:, 0:128], z_t[:, :],
                             start=True, stop=False)

            for seg in range(SEGS_PER_CORE):
                off = 32 * seg
                seg_pr = seg * PR_SEG
                for base, part in SEG_CHUNKS:
                    xt = xin_pool.tile([128, 512], BF16, tag="x")
                    nc.sync.dma_start(
                        xt[0:part, :], xb[seg_pr + base: seg_pr + base + part, :])
                    # transpose x slices: XtP[32w4+c, 128*j+p] (col base 128j
                    # keeps PSUM writes 4B-aligned even when part=53)
                    xtp = xtp_pool.tile([128, 512], BF16, tag="xtp")
                    for j in range(4):
                        nc.tensor.transpose(
                            xtp[:, 128 * j: 128 * j + part],
                            xt[0:part, 128 * j: 128 * j + 128],
                            id_t[0:part, 0:part])
                    xts = xts_pool.tile([128, 512], BF16, tag="xts")
                    hp = hp_pool.tile([64, 512], F32, tag="h")
                    hs = hs_pool.tile([64, 512], BF16, tag="hr")
                    aps = apsum_pool.tile([4, 512], F32, tag="a")
                    as_t = as_pool.tile([4, 512], BF16, tag="as")
                    if part == 128:
                        spans = [(0, 512)]
                    else:
                        spans = [(128 * j, 128 * j + part) for j in range(4)]
                    for lo, hi in spans:
                        nc.scalar.copy(xts[:, lo:hi], xtp[:, lo:hi])
                        nc.tensor.matmul(hp[:, lo:hi], w1_t[:], xts[:, lo:hi],
                                         start=True, stop=True)
                        nc.scalar.activation(hs[:, lo:hi], hp[:, lo:hi],
                                             mybir.ActivationFunctionType.Relu,
                                             bias=b1_t[:])
                        nc.tensor.matmul(aps[:, lo:hi], w2_t[:], hs[:, lo:hi],
                                         start=True, stop=True)
                        nc.vector.tensor_copy(as_t[:, lo:hi], aps[:, lo:hi])
                    # transpose A back: At[p, 4j+d]
                    atp = atp_pool.tile([128, 16], BF16, tag="atp")
                    for j in range(4):
                        nc.tensor.transpose(
                            atp[0:part, 4 * j: 4 * j + 4],
                            as_t[:, 128 * j: 128 * j + part],
                            id4_t[:])
                    ats = ats_pool.tile([128, 16], BF16, tag="ats")
                    nc.vector.tensor_copy(ats[0:part, :], atp[0:part, :])
                    at2 = ats_pool.tile([128, 16], BF16, tag="at2")
                    nc.vector.tensor_mul(at2[0:part, :], ats[0:part, :],
                                         ats[0:part, :])
                    # P/Q/Sa/Sa2 accumulate
                    for j in range(4):
                        nc.tensor.matmul(
                            acc[off:off + 4, 0:128],
                            ats[0:part, 4 * j:4 * j + 4],
                            xt[0:part, 128 * j:128 * j + 128],
                            start=False, stop=False, tile_position=(0, off))
                    nc.tensor.matmul(acc[64 + off:65 + off, 0:512],
                                     ones_t[0:part, :], xt[0:part, :],
                                     start=False, stop=False,
                                     tile_position=(0, 64 + off))
                    nc.tensor.matmul(acc[off:off + 1, 384:400],
                                     ones_t[0:part, :], ats[0:part, :],
                                     start=False, stop=False,
                                     tile_position=(0, off))
                    nc.tensor.matmul(acc[off:off + 1, 400:416],
                                     ones_t[0:part, :], at2[0:part, :],
                                     start=False, stop=False,
                                     tile_position=(0, off))
            outt = out_pool.tile([128, 512], F32)
            nc.scalar.copy(outt[:], acc[:])
            nc.sync.dma_start(oACC[:], outt[:])
    nc.compile()
    return nc


QCHUNK = 512          # quads per full main-pass chunk (2048 rows)
SEG_Q = PTS // 4      # 12500 quads per segment
FULL_CHUNKS = 24      # 24*512 quads; tail = 212 quads = 848 rows (53 pr)


def _build_main2():
    """v2: DMA-transposed quad-view main pass; v1-style PE-transpose tail."""
    nc = bacc.Bacc("TRN2", target_bir_lowering=False, debug=False,
                   num_devices=NCORES)
    xb = nc.dram_tensor("xb", [R // 4, 128], BF16, kind="ExternalInput").ap()
    W1s = nc.dram_tensor("W1s", [128, 64], BF16, kind="ExternalInput").ap()
    b1v = nc.dram_tensor("b1v", [64, 1], F32, kind="ExternalInput").ap()
    W2s = nc.dram_tensor("W2s", [64, 4], BF16, kind="ExternalInput").ap()
    ident = nc.dram_tensor("ident", [128, 128], BF16, kind="ExternalInput").ap()
    ident4 = nc.dram_tensor("ident4", [4, 4], BF16, kind="ExternalInput").ap()
    ones = nc.dram_tensor("ones", [128, 1], BF16, kind="ExternalInput").ap()
    zeros = nc.dram_tensor("zeros", [1, 512], BF16, kind="ExternalInput").ap()
    oACC = nc.dram_tensor("oACC", [128, 512], F32, kind="ExternalOutput").ap()
    oACC2 = nc.dram_tensor("oACC2", [128, 512], F32, kind="ExternalOutput").ap()

    xb16 = xb.rearrange("(p k) c -> p (k c)", k=4)  # [R//16, 512] natural view

    with tile.TileContext(nc) as tc:
        with (
            tc.tile_pool(name="consts", bufs=1) as cpool,
            tc.tile_pool(name="xT", bufs=3) as xT_pool,
            tc.tile_pool(name="xq", bufs=3) as xq_pool,
            tc.tile_pool(name="hp", bufs=2, space="PSUM") as hp_pool,
            tc.tile_pool(name="hs", bufs=2) as hs_pool,
            tc.tile_pool(name="ap", bufs=2, space="PSUM") as apsum_pool,
            tc.tile_pool(name="as_", bufs=2) as as_pool,
            tc.tile_pool(name="atp", bufs=1, space="PSUM") as atp_pool,
            tc.tile_pool(name="ats", bufs=2) as ats_pool,
            tc.tile_pool(name="xtp", bufs=1, space="PSUM") as xtp_pool,
            tc.tile_pool(name="acc", bufs=1, space="PSUM") as acc_pool,
            tc.tile_pool(name="outs", bufs=1) as out_pool,
        ):
            w1_t = cpool.tile([128, 64], BF16)
            nc.sync.dma_start(w1_t[:], W1s[:])
            b1_t = cpool.tile([64, 1], F32)
            nc.sync.dma_start(b1_t[:], b1v[:])
            w2_t = cpool.tile([64, 4], BF16)
            nc.sync.dma_start(w2_t[:], W2s[:])
            id_t = cpool.tile([128, 128], BF16)
            nc.sync.dma_start(id_t[:], ident[:])
            id4_t = cpool.tile([4, 4], BF16)
            nc.sync.dma_start(id4_t[:], ident4[:])
            ones_t = cpool.tile([128, 1], BF16)
            nc.sync.dma_start(ones_t[:], ones[:])
            z_t = cpool.tile([1, 512], BF16)
            nc.sync.dma_start(z_t[:], zeros[:])

            acc = acc_pool.tile([128, 512], F32, tag="acc")
            acc2 = acc_pool.tile([128, 512], F32, tag="acc2")
            nc.tensor.matmul(acc[:, :], z_t[:, 0:128], z_t[:, :],
                             start=True, stop=False)
            nc.tensor.matmul(acc2[:, :], z_t[:, 0:128], z_t[:, :],
                             start=True, stop=False)

            def mlp(xts_ap, part4, hp, hs, aps, as_t):
                """xts_ap: [128, part4] transposed input (SBUF bf16)."""
                nc.tensor.matmul(hp[:, 0:part4], w1_t[:], xts_ap,
                                 start=True, stop=True)
                nc.scalar.activation(hs[:, 0:part4], hp[:, 0:part4],
                                     mybir.ActivationFunctionType.Relu,
                                     bias=b1_t[:])
                nc.tensor.matmul(aps[:, 0:part4], w2_t[:], hs[:, 0:part4],
                                 start=True, stop=True)
                nc.vector.tensor_copy(as_t[:, 0:part4], aps[:, 0:part4])

            for seg in range(SEGS_PER_CORE):
                off = 32 * seg
                seg_q0 = seg * SEG_Q
                for tp in range(FULL_CHUNKS // 2):
                    q0p = seg_q0 + tp * 2 * QCHUNK
                    pr0p = q0p // 4
                    # paired 256KB transfers: one DMA-transpose + one natural
                    # 3D-AP load cover two 2048-row chunks each
                    xT2 = xT_pool.tile([128, 1024], BF16, tag="xT")
                    nc.sync.dma_start(xT2[:], xb[q0p:q0p + 2 * QCHUNK, :],
                                      transpose=True)
                    xq2 = xq_pool.tile([128, 1024], BF16, tag="xq")
                    nc.scalar.dma_start(
                        xq2[:, :].rearrange("p (e v) -> p e v", e=2),
                        xb[4 * pr0p: 4 * pr0p + 1024, :].rearrange(
                            "(e p k) c -> p e (k c)", e=2, k=4))
                    for e in range(2):
                        xT = xT2[:, 512 * e: 512 * e + 512]
                        xq = xq2[:, 512 * e: 512 * e + 512]
                        hp = hp_pool.tile([64, 512], F32, tag="h")
                        hs = hs_pool.tile([64, 512], BF16, tag="hr")
                        aps = apsum_pool.tile([4, 512], F32, tag="a")
                        as_t = as_pool.tile([4, 512], BF16, tag="as")
                        mlp(xT, 512, hp, hs, aps, as_t)
                        # bridge quad-order A -> 16-row natural order via
                        # stride-4 column slices: At[p, 4j+d] = As[d, 4p+j]
                        as3 = as_t[:, :].rearrange("g (p j) -> g j p", j=4)
                        atp = atp_pool.tile([128, 16], BF16, tag="atp")
                        for j in range(4):
                            nc.tensor.transpose(
                                atp[:, 4 * j: 4 * j + 4],
                                as3[:, j, :],
                                id4_t[:])
                        ats = ats_pool.tile([128, 16], BF16, tag="ats")
                        nc.vector.tensor_copy(ats[:, :], atp[:, :])
                        at2 = ats_pool.tile([128, 16], BF16, tag="at2")
                        nc.vector.tensor_mul(at2[:, :], ats[:, :], ats[:, :])
                        nc.tensor.matmul(acc[off:off + 16, 0:512], ats[:, :],
                                         xq, start=False, stop=False,
                                         tile_position=(0, off))
                        nc.tensor.matmul(acc[64 + off:65 + off, 0:512],
                                         ones_t[:, :], xq,
                                         start=False, stop=False,
                                         tile_position=(0, 64 + off))
                        nc.tensor.matmul(acc2[off:off + 1, 0:16],
                                         ones_t[:, :], ats[:, :],
                                         start=False, stop=False,
                                         tile_position=(0, off))
                        nc.tensor.matmul(acc2[off:off + 1, 16:32],
                                         ones_t[:, :], at2[:, :],
                                         start=False, stop=False,
                                         tile_position=(0, off))
                # ---- tail: 848 rows via v1 PE-transpose path ----
                part = 53
                pr0 = (seg * PTS + FULL_CHUNKS * QCHUNK * 4) // 16
                xt = xq_pool.tile([128, 512], BF16, tag="xq")
                nc.sync.dma_start(xt[0:part, :], xb16[pr0:pr0 + part, :])
                xtp = xtp_pool.tile([128, 512], BF16, tag="xtp")
                for j in range(4):
                    nc.tensor.transpose(
                        xtp[:, 128 * j: 128 * j + part],
                        xt[0:part, 128 * j: 128 * j + 128],
                        id_t[0:part, 0:part])
                xts = xT_pool.tile([128, 512], BF16, tag="xT")
                hp = hp_pool.tile([64, 512], F32, tag="h")
                hs = hs_pool.tile([64, 512], BF16, tag="hr")
                aps = apsum_pool.tile([4, 512], F32, tag="a")
                as_t = as_pool.tile([4, 512], BF16, tag="as")
                for j in range(4):
                    lo, hi = 128 * j, 128 * j + part
                    nc.scalar.copy(xts[:, lo:hi], xtp[:, lo:hi])
                    nc.tensor.matmul(hp[:, lo:hi], w1_t[:], xts[:, lo:hi],
                                     start=True, stop=True)
                    nc.scalar.activation(hs[:, lo:hi], hp[:, lo:hi],
                                         mybir.ActivationFunctionType.Relu,
                                         bias=b1_t[:])
                    nc.tensor.matmul(aps[:, lo:hi], w2_t[:], hs[:, lo:hi],
                                     start=True, stop=True)
                    nc.vector.tensor_copy(as_t[:, lo:hi], aps[:, lo:hi])
                atp = atp_pool.tile([128, 16], BF16, tag="atp")
                for j in range(4):
                    nc.tensor.transpose(
                        atp[0:part, 4 * j: 4 * j + 4],
                        as_t[:, 128 * j: 128 * j + part],
                        id4_t[:])
                ats = ats_pool.tile([128, 16], BF16, tag="ats")
                nc.vector.tensor_copy(ats[0:part, :], atp[0:part, :])
                at2 = ats_pool.tile([128, 16], BF16, tag="at2")
                nc.vector.tensor_mul(at2[0:part, :], ats[0:part, :],
                                     ats[0:part, :])
                for j in range(4):
                    nc.tensor.matmul(
                        acc2[off:off + 4, 32:160],
                        ats[0:part, 4 * j:4 * j + 4],
                        xt[0:part, 128 * j:128 * j + 128],
                        start=False, stop=False, tile_position=(0, off))
                nc.tensor.matmul(acc[64 + off:65 + off, 0:512],
                                 ones_t[0:part, :], xt[0:part, :],
                                 start=False, stop=False,
                                 tile_position=(0, 64 + off))
                nc.tensor.matmul(acc2[off:off + 1, 0:16],
                                 ones_t[0:part, :], ats[0:part, :],
                                 start=False, stop=False,
                                 tile_position=(0, off))
                nc.tensor.matmul(acc2[off:off + 1, 16:32],
                                 ones_t[0:part, :], at2[0:part, :],
                                 start=False, stop=False,
                                 tile_position=(0, off))
            outt = out_pool.tile([128, 512], F32, tag="o1")
            nc.scalar.copy(outt[:], acc[:])
            nc.sync.dma_start(oACC[:], outt[:])
            outt2 = out_pool.tile([128, 512], F32, tag="o2")
            nc.scalar.copy(outt2[:], acc2[:])
            nc.sync.dma_start(oACC2[:], outt2[:])
    nc.compile()
    return nc


_NC_CACHE = {}


def _get_nc(name):
    if name not in _NC_CACHE:
        _NC_CACHE[name] = _build_gram() if name == "gram" else _build_main2()
    return _NC_CACHE[name]


def kernel(**inputs):
    x = np.asarray(inputs["x"], np.float32)
    W1 = np.asarray(inputs["W1"], np.float32)
    b1 = np.asarray(inputs["b1"], np.float64)
    g1 = np.asarray(inputs["gamma1"], np.float64)
    be1 = np.asarray(inputs["beta1"], np.float64)
    W2 = np.asarray(inputs["W2"], np.float32)
    b2 = np.asarray(inputs["b2"], np.float64)
    g2 = np.asarray(inputs["gamma2"], np.float64)
    be2 = np.asarray(inputs["beta2"], np.float64)
    length = np.asarray(inputs["length"], np.float32)

    N = x.shape[0]
    assert N == N_TOTAL
    xb = x.astype(BF)
    xb_cores = np.ascontiguousarray(xb.reshape(NCORES, R // 16, 512))

    ones_np = np.ones((128, 1), BF)
    core_ids = list(range(NCORES))

    # ---- launch 1: Gram ----
    nc1 = _get_nc("gram")
    in_maps1 = [{"xb": xb_cores[k], "ones": ones_np} for k in core_ids]
    res1 = run_bass_kernel_spmd(nc1, in_maps1, core_ids).results
    G = np.zeros((128, 129), np.float64)
    for k in core_ids:
        G += res1[k]["oG"]
    xtx = np.zeros((32, 32), np.float64)
    sx = np.zeros(32, np.float64)
    for d in range(4):
        xtx += G[32 * d:32 * d + 32, 32 * d:32 * d + 32]
        sx += G[32 * d:32 * d + 32, 128]
    mean = sx / N
    C = xtx / N - np.outer(mean, mean)
    W1d = W1.astype(np.float64)
    var_h = np.einsum('jc,cd,jd->j', W1d, C, W1d)
    m_h = W1d @ mean + b1
    s1 = g1 / np.sqrt(var_h + EPS_BN)
    W1fold = (W1d * s1[:, None])
    b1fold = (s1 * (b1 - m_h) + be1)

    W1s_np = np.zeros((128, 64), np.float32)
    b1v_np = np.zeros((64, 1), np.float32)
    W2s_np = np.zeros((64, 4), np.float32)
    for w4 in range(4):
        W1s_np[32 * w4:32 * w4 + 32, 16 * w4:16 * w4 + 16] = W1fold.T
        b1v_np[16 * w4:16 * w4 + 16, 0] = b1fold
        W2s_np[16 * w4:16 * w4 + 16, w4] = W2[0]
    ident_np = np.eye(128, dtype=BF)
    ident4_np = np.eye(4, dtype=BF)
    zeros_np = np.zeros((1, 512), BF)

    # ---- launch 2: main (v2 layout) ----
    nc2 = _get_nc("main")
    common = {"W1s": W1s_np.astype(BF), "b1v": b1v_np,
              "W2s": W2s_np.astype(BF), "ident": ident_np,
              "ident4": ident4_np, "ones": ones_np, "zeros": zeros_np}
    xb_cores4 = xb_cores.reshape(NCORES, R // 4, 128)
    in_maps2 = [{"xb": xb_cores4[k], **common} for k in core_ids]
    res2 = run_bass_kernel_spmd(nc2, in_maps2, core_ids).results

    P = np.zeros((16, 32), np.float64)
    Q = np.zeros((16, 32), np.float64)
    Sa = 0.0
    Sa2 = 0.0
    for k in core_ids:
        ACC = res2[k]["oACC"].astype(np.float64)
        ACC2 = res2[k]["oACC2"].astype(np.float64)
        for seg in range(SEGS_PER_CORE):
            off = 32 * seg
            s = SEGS_PER_CORE * k + seg
            for t in range(4):
                for g in range(4):
                    P[s] += ACC[off + 4 * t + g,
                                128 * t + 32 * g: 128 * t + 32 * g + 32]
            for d in range(4):
                P[s] += ACC2[off + d, 32 + 32 * d: 64 + 32 * d]
            Q[s] += ACC[64 + off, 0:512].reshape(16, 32).sum(axis=0)
            Sa += ACC2[off, 0:16].sum()
            Sa2 += ACC2[off, 16:32].sum()

    b2d = float(b2[0])
    m2 = (Sa + N * b2d) / N
    e2 = (Sa2 + 2 * b2d * Sa + N * b2d * b2d) / N
    v2 = e2 - m2 * m2
    s2 = float(g2[0]) / np.sqrt(v2 + EPS_BN)
    seg_sum = s2 * P + (s2 * (b2d - m2) + float(be2[0])) * Q
    result = seg_sum / length.astype(np.float64)[:, None]
    norm = np.linalg.norm(result, axis=1, keepdims=True)
    out = result / np.maximum(norm, EPS_NORM)
    return out.astype(np.float32)


# revision 10
# speedup vs baseline: 20485.2149x; 1.4050x over previous
"""Trainium2 Bass kernel for nn_FCGF_point_att3 (segment_reduce).

Pipeline (per reference.py):
  h = x@W1.T + b1 ; h = relu(BN(h)) ; a = BN(h@W2.T + b2)
  out = l2norm(segment_mean(x * a))   with global (all-N) BN stats.

Strategy: 8-way data parallel over segments (2 segments of 50k points per
core).  Two SPMD launches:
  L1: per-core Gram matrix G = [X|1]^T[X|1] in bf16 on the PE via the
      "reinterp" trick (rows on the contraction axis, no transpose needed).
      Host combines G across cores -> exact global BN1 stats -> folds BN1
      into W1,b1.
  L2: per-core main pass: PE-transpose x tiles, stacked-blockdiag MLP
      (32->16->1) on the PE, per-segment P = sum(x*a~), Q = sum(x),
      Sa = sum(a~), Sa2 = sum(a~^2) accumulated in PSUM.
      Host applies BN2 as an affine post-correction:
      seg_sum = s2*P + (s2*(b2-m2)+beta2)*Q, then mean + L2 normalize.
"""

import numpy as np
import ml_dtypes

import concourse.bass as bass
import concourse.tile as tile
from concourse import bacc, mybir
from concourse.bass_utils import run_bass_kernel_spmd

BF = ml_dtypes.bfloat16
F32 = mybir.dt.float32
BF16 = mybir.dt.bfloat16

NCORES = 8
PTS = 50000          # points per segment
SEGS_PER_CORE = 2
R = PTS * SEGS_PER_CORE   # rows per core
CIN = 32
CH = 16
N_TOTAL = NCORES * R
EPS_BN = 1e-5
EPS_NORM = 1e-12

PR_SEG = PTS // 16         # 3125 partition-rows per segment (16 rows each)
CHUNK_PR = 128             # partition-rows per full chunk
SEG_CHUNKS = [(t * CHUNK_PR, min(CHUNK_PR, PR_SEG - t * CHUNK_PR))
              for t in range((PR_SEG + CHUNK_PR - 1) // CHUNK_PR)]  # 24x128 + 53


def _build_gram():
    nc = bacc.Bacc("TRN2", target_bir_lowering=False, debug=False,
                   num_devices=NCORES)
    xb = nc.dram_tensor("xb", [R // 16, 512], BF16, kind="ExternalInput").ap()
    ones = nc.dram_tensor("ones", [128, 1], BF16, kind="ExternalInput").ap()
    oG = nc.dram_tensor("oG", [128, 129], F32, kind="ExternalOutput").ap()

    # pair rows: [3125, 1024] view, 128-partition tiles hold 4096 rows each
    xb2 = xb.rearrange("(a b) c -> a (b c)", b=2)
    n_pr2 = R // 32  # 3125
    chunks = [(t * CHUNK_PR, min(CHUNK_PR, n_pr2 - t * CHUNK_PR))
              for t in range((n_pr2 + CHUNK_PR - 1) // CHUNK_PR)]

    with tile.TileContext(nc) as tc:
        with (
            tc.tile_pool(name="xin", bufs=4) as xin_pool,
            tc.tile_pool(name="consts", bufs=1) as cpool,
            tc.tile_pool(name="accp", bufs=1, space="PSUM") as acc_pool,
            tc.tile_pool(name="outs", bufs=1) as out_pool,
        ):
            ones_t = cpool.tile([128, 1], BF16)
            nc.sync.dma_start(ones_t[:], ones[:])
            acc = acc_pool.tile([128, 129], F32)  # G | S
            first = True
            for base, part in chunks:
                xt = xin_pool.tile([128, 1024], BF16, tag="x")
                nc.sync.dma_start(xt[0:part, :], xb2[base:base + part, :])
                for j in range(8):
                    sl = xt[0:part, 128 * j:128 * j + 128]
                    nc.tensor.matmul(acc[:, 0:128], sl, sl,
                                     start=first, stop=False)
                    first = False
                    nc.tensor.matmul(acc[:, 128:129], sl, ones_t[0:part, :],
                                     start=False, stop=False)
            outt = out_pool.tile([128, 129], F32)
            nc.scalar.copy(outt[:], acc[:])
            nc.sync.dma_start(oG[:], outt[:])
    nc.compile()
    return nc


def _build_main():
    nc = bacc.Bacc("TRN2", target_bir_lowering=False, debug=False,
                   num_devices=NCORES)
    xb = nc.dram_tensor("xb", [R // 16, 512], BF16, kind="ExternalInput").ap()
    W1s = nc.dram_tensor("W1s", [128, 64], BF16, kind="ExternalInput").ap()
    b1v = nc.dram_tensor("b1v", [64, 1], F32, kind="ExternalInput").ap()
    W2s = nc.dram_tensor("W2s", [64, 4], BF16, kind="ExternalInput").ap()
    ident = nc.dram_tensor("ident", [128, 128], BF16, kind="ExternalInput").ap()
    ident4 = nc.dram_tensor("ident4", [4, 4], BF16, kind="ExternalInput").ap()
    ones = nc.dram_tensor("ones", [128, 1], BF16, kind="ExternalInput").ap()
    zeros = nc.dram_tensor("zeros", [1, 512], BF16, kind="ExternalInput").ap()
    oACC = nc.dram_tensor("oACC", [128, 512], F32, kind="ExternalOutput").ap()

    with tile.TileContext(nc) as tc:
        with (
            tc.tile_pool(name="consts", bufs=1) as cpool,
            tc.tile_pool(name="xin", bufs=3) as xin_pool,
            tc.tile_pool(name="xtp", bufs=2, space="PSUM") as xtp_pool,
            tc.tile_pool(name="xts", bufs=2) as xts_pool,
            tc.tile_pool(name="hp", bufs=2, space="PSUM") as hp_pool,
            tc.tile_pool(name="hs", bufs=2) as hs_pool,
            tc.tile_pool(name="ap", bufs=1, space="PSUM") as apsum_pool,
            tc.tile_pool(name="as_", bufs=2) as as_pool,
            tc.tile_pool(name="atp", bufs=1, space="PSUM") as atp_pool,
            tc.tile_pool(name="ats", bufs=2) as ats_pool,
            tc.tile_pool(name="accp", bufs=1, space="PSUM") as acc_pool,
            tc.tile_pool(name="outs", bufs=1) as out_pool,
        ):
            w1_t = cpool.tile([128, 64], BF16)
            nc.sync.dma_start(w1_t[:], W1s[:])
            b1_t = cpool.tile([64, 1], F32)
            nc.sync.dma_start(b1_t[:], b1v[:])
            w2_t = cpool.tile([64, 4], BF16)
            nc.sync.dma_start(w2_t[:], W2s[:])
            id_t = cpool.tile([128, 128], BF16)
            nc.sync.dma_start(id_t[:], ident[:])
            id4_t = cpool.tile([4, 4], BF16)
            nc.sync.dma_start(id4_t[:], ident4[:])
            ones_t = cpool.tile([128, 1], BF16)
            nc.sync.dma_start(ones_t[:], ones[:])
            z_t = cpool.tile([1, 512], BF16)
            nc.sync.dma_start(z_t[:], zeros[:])

            acc = acc_pool.tile([128, 512], F32)
            # open one accumulation group covering the whole bank
            nc.tensor.matmul(acc[:, :], z_t[:, 0:128], z_t[:, :],
                             start=True, stop=False)

            for seg in range(SEGS_PER_CORE):
                off = 32 * seg
                seg_pr = seg * PR_SEG
                for base, part in SEG_CHUNKS:
                    xt = xin_pool.tile([128, 512], BF16, tag="x")
                    nc.sync.dma_start(
                        xt[0:part, :], xb[seg_pr + base: seg_pr + base + part, :])
                    # transpose x slices: XtP[32w4+c, 128*j+p] (col base 128j
                    # keeps PSUM writes 4B-aligned even when part=53)
                    xtp = xtp_pool.tile([128, 512], BF16, tag="xtp")
                    for j in range(4):
                        nc.tensor.transpose(
                            xtp[:, 128 * j: 128 * j + part],
                            xt[0:part, 128 * j: 128 * j + 128],
                            id_t[0:part, 0:part])
                    xts = xts_pool.tile([128, 512], BF16, tag="xts")
                    hp = hp_pool.tile([64, 512], F32, tag="h")
                    hs = hs_pool.tile([64, 512], BF16, tag="hr")
                    aps = apsum_pool.tile([4, 512], F32, tag="a")
                    as_t = as_pool.tile([4, 512], BF16, tag="as")
                    if part == 128:
                        spans = [(0, 512)]
                    else:
                        spans = [(128 * j, 128 * j + part) for j in range(4)]
                    for lo, hi in spans:
                        nc.scalar.copy(xts[:, lo:hi], xtp[:, lo:hi])
                        nc.tensor.matmul(hp[:, lo:hi], w1_t[:], xts[:, lo:hi],
                                         start=True, stop=True)
                        nc.scalar.activation(hs[:, lo:hi], hp[:, lo:hi],
                                             mybir.ActivationFunctionType.Relu,
                                             bias=b1_t[:])
                        nc.tensor.matmul(aps[:, lo:hi], w2_t[:], hs[:, lo:hi],
                                         start=True, stop=True)
                        nc.vector.tensor_copy(as_t[:, lo:hi], aps[:, lo:hi])
                    # transpose A back: At[p, 4j+d]
                    atp = atp_pool.tile([128, 16], BF16, tag="atp")
                    for j in range(4):
                        nc.tensor.transpose(
                            atp[0:part, 4 * j: 4 * j + 4],
                            as_t[:, 128 * j: 128 * j + part],
                            id4_t[:])
                    ats = ats_pool.tile([128, 16], BF16, tag="ats")
                    nc.vector.tensor_copy(ats[0:part, :], atp[0:part, :])
                    at2 = ats_pool.tile([128, 16], BF16, tag="at2")
                    nc.vector.tensor_mul(at2[0:part, :], ats[0:part, :],
                                         ats[0:part, :])
                    # P/Q/Sa/Sa2 accumulate
                    for j in range(4):
                        nc.tensor.matmul(
                            acc[off:off + 4, 0:128],
                            ats[0:part, 4 * j:4 * j + 4],
                            xt[0:part, 128 * j:128 * j + 128],
                            start=False, stop=False, tile_position=(0, off))
                    nc.tensor.matmul(acc[64 + off:65 + off, 0:512],
                                     ones_t[0:part, :], xt[0:part, :],
                                     start=False, stop=False,
                                     tile_position=(0, 64 + off))
                    nc.tensor.matmul(acc[off:off + 1, 384:400],
                                     ones_t[0:part, :], ats[0:part, :],
                                     start=False, stop=False,
                                     tile_position=(0, off))
                    nc.tensor.matmul(acc[off:off + 1, 400:416],
                                     ones_t[0:part, :], at2[0:part, :],
                                     start=False, stop=False,
                                     tile_position=(0, off))
            outt = out_pool.tile([128, 512], F32)
            nc.scalar.copy(outt[:], acc[:])
            nc.sync.dma_start(oACC[:], outt[:])
    nc.compile()
    return nc


QCHUNK = 512          # quads per full main-pass chunk (2048 rows)
SEG_Q = PTS // 4      # 12500 quads per segment
FULL_CHUNKS = 24      # 24*512 quads; tail = 212 quads = 848 rows (53 pr)


def _build_main2():
    """v2: DMA-transposed quad-view main pass; v1-style PE-transpose tail."""
    nc = bacc.Bacc("TRN2", target_bir_lowering=False, debug=False,
                   num_devices=NCORES)
    xb = nc.dram_tensor("xb", [R // 4, 128], BF16, kind="ExternalInput").ap()
    W1s = nc.dram_tensor("W1s", [128, 64], BF16, kind="ExternalInput").ap()
    b1v = nc.dram_tensor("b1v", [64, 1], F32, kind="ExternalInput").ap()
    W2s = nc.dram_tensor("W2s", [64, 4], BF16, kind="ExternalInput").ap()
    ident = nc.dram_tensor("ident", [128, 128], BF16, kind="ExternalInput").ap()
    ident4 = nc.dram_tensor("ident4", [4, 4], BF16, kind="ExternalInput").ap()
    ones = nc.dram_tensor("ones", [128, 1], BF16, kind="ExternalInput").ap()
    zeros = nc.dram_tensor("zeros", [1, 512], BF16, kind="ExternalInput").ap()
    oACC = nc.dram_tensor("oACC", [128, 512], F32, kind="ExternalOutput").ap()
    oACC2 = nc.dram_tensor("oACC2", [128, 512], F32, kind="ExternalOutput").ap()

    xb16 = xb.rearrange("(p k) c -> p (k c)", k=4)  # [R//16, 512] natural view

    with tile.TileContext(nc) as tc:
        with (
            tc.tile_pool(name="consts", bufs=1) as cpool,
            tc.tile_pool(name="xT", bufs=3) as xT_pool,
            tc.tile_pool(name="xq", bufs=3) as xq_pool,
            tc.tile_pool(name="hp", bufs=2, space="PSUM") as hp_pool,
            tc.tile_pool(name="hs", bufs=2) as hs_pool,
            tc.tile_pool(name="ap", bufs=2, space="PSUM") as apsum_pool,
            tc.tile_pool(name="as_", bufs=2) as as_pool,
            tc.tile_pool(name="atp", bufs=1, space="PSUM") as atp_pool,
            tc.tile_pool(name="ats", bufs=2) as ats_pool,
            tc.tile_pool(name="xtp", bufs=1, space="PSUM") as xtp_pool,
            tc.tile_pool(name="acc", bufs=1, space="PSUM") as acc_pool,
            tc.tile_pool(name="outs", bufs=1) as out_pool,
        ):
            w1_t = cpool.tile([128, 64], BF16)
            nc.sync.dma_start(w1_t[:], W1s[:])
            b1_t = cpool.tile([64, 1], F32)
            nc.sync.dma_start(b1_t[:], b1v[:])
            w2_t = cpool.tile([64, 4], BF16)
            nc.sync.dma_start(w2_t[:], W2s[:])
            id_t = cpool.tile([128, 128], BF16)
            nc.sync.dma_start(id_t[:], ident[:])
            id4_t = cpool.tile([4, 4], BF16)
            nc.sync.dma_start(id4_t[:], ident4[:])
            ones_t = cpool.tile([128, 1], BF16)
            nc.sync.dma_start(ones_t[:], ones[:])
            z_t = cpool.tile([1, 512], BF16)
            nc.sync.dma_start(z_t[:], zeros[:])

            acc = acc_pool.tile([128, 512], F32, tag="acc")
            acc2 = acc_pool.tile([128, 512], F32, tag="acc2")
            nc.tensor.matmul(acc[:, :], z_t[:, 0:128], z_t[:, :],
                             start=True, stop=False)
            nc.tensor.matmul(acc2[:, :], z_t[:, 0:128], z_t[:, :],
                             start=True, stop=False)

            def mlp(xts_ap, part4, hp, hs, aps, as_t):
                """xts_ap: [128, part4] transposed input (SBUF bf16)."""
                nc.tensor.matmul(hp[:, 0:part4], w1_t[:], xts_ap,
                                 start=True, stop=True)
                nc.scalar.activation(hs[:, 0:part4], hp[:, 0:part4],
                                     mybir.ActivationFunctionType.Relu,
                                     bias=b1_t[:])
                nc.tensor.matmul(aps[:, 0:part4], w2_t[:], hs[:, 0:part4],
                                 start=True, stop=True)
                nc.vector.tensor_copy(as_t[:, 0:part4], aps[:, 0:part4])

            for seg in range(SEGS_PER_CORE):
                off = 32 * seg
                seg_q0 = seg * SEG_Q
                for tp in range(FULL_CHUNKS // 4):
                    q0p = seg_q0 + tp * 4 * QCHUNK
                    pr0p = q0p // 4
                    # grouped 512KB transfers: one DMA-transpose + one natural
                    # 3D-AP load cover four 2048-row chunks each
                    xT2 = xT_pool.tile([128, 2048], BF16, tag="xT")
                    nc.sync.dma_start(xT2[:], xb[q0p:q0p + 4 * QCHUNK, :],
                                      transpose=True)
                    xq2 = xq_pool.tile([128, 2048], BF16, tag="xq")
                    nc.scalar.dma_start(
                        xq2[:, :].rearrange("p (e v) -> p e v", e=4),
                        xb[4 * pr0p: 4 * pr0p + 2048, :].rearrange(
                            "(e p k) c -> p e (k c)", e=4, k=4))
                    for e in range(4):
                        xT = xT2[:, 512 * e: 512 * e + 512]
                        xq = xq2[:, 512 * e: 512 * e + 512]
                        hp = hp_pool.tile([64, 512], F32, tag="h")
                        hs = hs_pool.tile([64, 512], BF16, tag="hr")
                        aps = apsum_pool.tile([4, 512], F32, tag="a")
                        as_t = as_pool.tile([4, 512], BF16, tag="as")
                        mlp(xT, 512, hp, hs, aps, as_t)
                        # bridge quad-order A -> 16-row natural order via
                        # stride-4 column slices: At[p, 4j+d] = As[d, 4p+j]
                        as3 = as_t[:, :].rearrange("g (p j) -> g j p", j=4)
                        atp = atp_pool.tile([128, 16], BF16, tag="atp")
                        for j in range(4):
                            nc.tensor.transpose(
                                atp[:, 4 * j: 4 * j + 4],
                                as3[:, j, :],
                                id4_t[:])
                        ats = ats_pool.tile([128, 16], BF16, tag="ats")
                        nc.vector.tensor_copy(ats[:, :], atp[:, :])
                        at2 = ats_pool.tile([128, 16], BF16, tag="at2")
                        nc.vector.tensor_mul(at2[:, :], ats[:, :], ats[:, :])
                        nc.tensor.matmul(acc[off:off + 16, 0:512], ats[:, :],
                                         xq, start=False, stop=False,
                                         tile_position=(0, off))
                        nc.tensor.matmul(acc[64 + off:65 + off, 0:512],
                                         ones_t[:, :], xq,
                                         start=False, stop=False,
                                         tile_position=(0, 64 + off))
                        nc.tensor.matmul(acc2[off:off + 1, 0:16],
                                         ones_t[:, :], ats[:, :],
                                         start=False, stop=False,
                                         tile_position=(0, off))
                        nc.tensor.matmul(acc2[off:off + 1, 16:32],
                                         ones_t[:, :], at2[:, :],
                                         start=False, stop=False,
                                         tile_position=(0, off))
                # ---- tail: 848 rows via v1 PE-transpose path ----
                part = 53
                pr0 = (seg * PTS + FULL_CHUNKS * QCHUNK * 4) // 16
                xt = xq_pool.tile([128, 512], BF16, tag="xq")
                nc.sync.dma_start(xt[0:part, :], xb16[pr0:pr0 + part, :])
                xtp = xtp_pool.tile([128, 512], BF16, tag="xtp")
                for j in range(4):
                    nc.tensor.transpose(
                        xtp[:, 128 * j: 128 * j + part],
                        xt[0:part, 128 * j: 128 * j + 128],
                        id_t[0:part, 0:part])
                xts = xT_pool.tile([128, 512], BF16, tag="xT")
                hp = hp_pool.tile([64, 512], F32, tag="h")
                hs = hs_pool.tile([64, 512], BF16, tag="hr")
                aps = apsum_pool.tile([4, 512], F32, tag="a")
                as_t = as_pool.tile([4, 512], BF16, tag="as")
                for j in range(4):
                    lo, hi = 128 * j, 128 * j + part
                    nc.scalar.copy(xts[:, lo:hi], xtp[:, lo:hi])
                    nc.tensor.matmul(hp[:, lo:hi], w1_t[:], xts[:, lo:hi],
                                     start=True, stop=True)
                    nc.scalar.activation(hs[:, lo:hi], hp[:, lo:hi],
                                         mybir.ActivationFunctionType.Relu,
                                         bias=b1_t[:])
                    nc.tensor.matmul(aps[:, lo:hi], w2_t[:], hs[:, lo:hi],
                                     start=True, stop=True)
                    nc.vector.tensor_copy(as_t[:, lo:hi], aps[:, lo:hi])
                atp = atp_pool.tile([128, 16], BF16, tag="atp")
                for j in range(4):
                    nc.tensor.transpose(
                        atp[0:part, 4 * j: 4 * j + 4],
                        as_t[:, 128 * j: 128 * j + part],
                        id4_t[:])
                ats = ats_pool.tile([128, 16], BF16, tag="ats")
                nc.vector.tensor_copy(ats[0:part, :], atp[0:part, :])
                at2 = ats_pool.tile([128, 16], BF16, tag="at2")
                nc.vector.tensor_mul(at2[0:part, :], ats[0:part, :],
                                     ats[0:part, :])
                for j in range(4):
                    nc.tensor.matmul(
                        acc2[off:off + 4, 32:160],
                        ats[0:part, 4 * j:4 * j + 4],
                        xt[0:part, 128 * j:128 * j + 128],
                        start=False, stop=False, tile_position=(0, off))
                nc.tensor.matmul(acc[64 + off:65 + off, 0:512],
                                 ones_t[0:part, :], xt[0:part, :],
                                 start=False, stop=False,
                                 tile_position=(0, 64 + off))
                nc.tensor.matmul(acc2[off:off + 1, 0:16],
                                 ones_t[0:part, :], ats[0:part, :],
                                 start=False, stop=False,
                                 tile_position=(0, off))
                nc.tensor.matmul(acc2[off:off + 1, 16:32],
                                 ones_t[0:part, :], at2[0:part, :],
                                 start=False, stop=False,
                                 tile_position=(0, off))
            outt = out_pool.tile([128, 512], F32, tag="o1")
            nc.scalar.copy(outt[:], acc[:])
            nc.sync.dma_start(oACC[:], outt[:])
            outt2 = out_pool.tile([128, 512], F32, tag="o2")
            nc.scalar.copy(outt2[:], acc2[:])
            nc.sync.dma_start(oACC2[:], outt2[:])
    nc.compile()
    return nc


_NC_CACHE = {}


def _get_nc(name):
    if name not in _NC_CACHE:
        _NC_CACHE[name] = _build_gram() if name == "gram" else _build_main2()
    return _NC_CACHE[name]


def kernel(**inputs):
    x = np.asarray(inputs["x"], np.float32)
    W1 = np.asarray(inputs["W1"], np.float32)
    b1 = np.asarray(inputs["b1"], np.float64)
    g1 = np.asarray(inputs["gamma1"], np.float64)
    be1 = np.asarray(inputs["beta1"], np.float64)
    W2 = np.asarray(inputs["W2"], np.float32)
    b2 = np.asarray(inputs["b2"], np.float64)
    g2 = np.asarray(inputs["gamma2"], np.float64)
    be2 = np.asarray(inputs["beta2"], np.float64)
    length = np.asarray(inputs["length"], np.float32)

    N = x.shape[0]
    assert N == N_TOTAL
    xb = x.astype(BF)
    xb_cores = np.ascontiguousarray(xb.reshape(NCORES, R // 16, 512))

    ones_np = np.ones((128, 1), BF)
    core_ids = list(range(NCORES))

    # ---- launch 1: Gram ----
    nc1 = _get_nc("gram")
    in_maps1 = [{"xb": xb_cores[k], "ones": ones_np} for k in core_ids]
    res1 = run_bass_kernel_spmd(nc1, in_maps1, core_ids).results
    G = np.zeros((128, 129), np.float64)
    for k in core_ids:
        G += res1[k]["oG"]
    xtx = np.zeros((32, 32), np.float64)
    sx = np.zeros(32, np.float64)
    for d in range(4):
        xtx += G[32 * d:32 * d + 32, 32 * d:32 * d + 32]
        sx += G[32 * d:32 * d + 32, 128]
    mean = sx / N
    C = xtx / N - np.outer(mean, mean)
    W1d = W1.astype(np.float64)
    var_h = np.einsum('jc,cd,jd->j', W1d, C, W1d)
    m_h = W1d @ mean + b1
    s1 = g1 / np.sqrt(var_h + EPS_BN)
    W1fold = (W1d * s1[:, None])
    b1fold = (s1 * (b1 - m_h) + be1)

    W1s_np = np.zeros((128, 64), np.float32)
    b1v_np = np.zeros((64, 1), np.float32)
    W2s_np = np.zeros((64, 4), np.float32)
    for w4 in range(4):
        W1s_np[32 * w4:32 * w4 + 32, 16 * w4:16 * w4 + 16] = W1fold.T
        b1v_np[16 * w4:16 * w4 + 16, 0] = b1fold
        W2s_np[16 * w4:16 * w4 + 16, w4] = W2[0]
    ident_np = np.eye(128, dtype=BF)
    ident4_np = np.eye(4, dtype=BF)
    zeros_np = np.zeros((1, 512), BF)

    # ---- launch 2: main (v2 layout) ----
    nc2 = _get_nc("main")
    common = {"W1s": W1s_np.astype(BF), "b1v": b1v_np,
              "W2s": W2s_np.astype(BF), "ident": ident_np,
              "ident4": ident4_np, "ones": ones_np, "zeros": zeros_np}
    xb_cores4 = xb_cores.reshape(NCORES, R // 4, 128)
    in_maps2 = [{"xb": xb_cores4[k], **common} for k in core_ids]
    res2 = run_bass_kernel_spmd(nc2, in_maps2, core_ids).results

    P = np.zeros((16, 32), np.float64)
    Q = np.zeros((16, 32), np.float64)
    Sa = 0.0
    Sa2 = 0.0
    for k in core_ids:
        ACC = res2[k]["oACC"].astype(np.float64)
        ACC2 = res2[k]["oACC2"].astype(np.float64)
        for seg in range(SEGS_PER_CORE):
            off = 32 * seg
            s = SEGS_PER_CORE * k + seg
            for t in range(4):
                for g in range(4):
                    P[s] += ACC[off + 4 * t + g,
                                128 * t + 32 * g: 128 * t + 32 * g + 32]
            for d in range(4):
                P[s] += ACC2[off + d, 32 + 32 * d: 64 + 32 * d]
            Q[s] += ACC[64 + off, 0:512].reshape(16, 32).sum(axis=0)
            Sa += ACC2[off, 0:16].sum()
            Sa2 += ACC2[off, 16:32].sum()

    b2d = float(b2[0])
    m2 = (Sa + N * b2d) / N
    e2 = (Sa2 + 2 * b2d * Sa + N * b2d * b2d) / N
    v2 = e2 - m2 * m2
    s2 = float(g2[0]) / np.sqrt(v2 + EPS_BN)
    seg_sum = s2 * P + (s2 * (b2d - m2) + float(be2[0])) * Q
    result = seg_sum / length.astype(np.float64)[:, None]
    norm = np.linalg.norm(result, axis=1, keepdims=True)
    out = result / np.maximum(norm, EPS_NORM)
    return out.astype(np.float32)
